# revision 1
# baseline (speedup 1.0000x reference)
"""Causal self-attention (B=2, T=2048, C=1024, 16 heads) on 8 TRN2 NeuronCores.

Sharding: 2-way data parallel (batch) x 4-way tensor parallel (heads).
Core c handles batch c//4 and heads [4*(c%4) .. 4*(c%4)+3].

Per-core pipeline (all matmuls bf16, fp32 PSUM accumulation):
  - host pre-transposes x[b] -> xT [C, T] bf16 so the contraction dim is
    on partitions everywhere (no on-device transposes needed).
  - q/k projections computed directly in transposed layout [j, T]
    (lhsT = weight columns, rhs = xT); Q^T, K^T per head are partition
    slices of the result.
  - v computed in natural [T, d] layout (lhsT = xT chunks, rhs = Wv),
    stored per (t-tile, head) as [128, 65] with a ones-column appended
    so the PV matmul also emits the softmax denominator for free.
  - attention: S^T tiles [kblock=128, qblock<=512] = K^T.T @ Q^T; exp on
    ScalarE (1/8 scale folded in; no max subtraction -- scores are O(1)
    by construction); causal masking is multiplicative post-exp on
    diagonal tiles only, restricted to their live column range;
    off-causal tiles are skipped.  O^T [65, qblock] accumulates over
    kblocks in PSUM.
  - y^T = O^T[0:64] * recip(O^T[64]) (GpSimd partition-broadcasts the
    reciprocal), written bf16 directly into the proj lhsT layout.
  - z_partial = y^T.T @ Wp_rows; AllReduce(add) over the 4 cores of the
    same batch, chunked to overlap with compute.

Self-contained: hardcodes shapes; only imports the system concourse stack.
"""

import contextlib

import numpy as np
import ml_dtypes

B, T, C = 2, 2048, 1024
NH = 16
HS = 64
NCORES = 8
HPC = 4          # heads per core
CPC = HPC * HS   # channels per core (256)
P = 128
QB = 512         # query block (free dim of S^T / O^T tiles)
NQ = T // QB     # 4 query blocks
NTT = T // P     # 16 t-tiles / kblocks
KC = C // P      # 8 contraction chunks
GROUPS = [[0, 1, 2, 3], [4, 5, 6, 7]]

_CACHE = {}

DEFAULT_CFG = dict(
    loop=1,          # repeat body (timing instrument)
    with_cc=True,    # AllReduce (False: plain DMA out, for TimelineSim)
    n_devices=NCORES,
    pair_exp=True,   # one [128,1024] exp per off-diagonal kblock pair
    tail_split=True, # last query block's output chunked per t-tile
    interleave=1,    # heads processed together in attention (1 or 2)
    merged=True,     # weave qkv/proj filler units into attention emission
    ppool_bufs=8,
    zpool_bufs=4,
    rpool_bufs=4,
    s_bufs=2,
    o_bufs=2,
    mm_bufs=2,
    weave_bias=2.0,  # <1: fillers front-loaded in each round; >1: back-loaded
    pe_bcast=False,  # broadcast softmax recip via PE ones-matmul vs GpSimd
    qi_first=1,      # first query block processed (rotation: 1 -> 1,2,3,0)
)


def _build_nc(cfg):
    import concourse.tile as tile
    import concourse.mybir as mybir
    from concourse import bacc

    f32 = mybir.dt.float32
    bf16 = mybir.dt.bfloat16
    Alu = mybir.AluOpType

    nc = bacc.Bacc(
        "TRN2",
        target_bir_lowering=False,
        debug=False,
        enable_asserts=True,
        num_devices=cfg["n_devices"],
    )
    aps = dict(
        xT=nc.dram_tensor("xT", [C, T], bf16, kind="ExternalInput").ap(),
        wqk=nc.dram_tensor("wqk", [C, 2 * CPC], bf16, kind="ExternalInput").ap(),
        wv=nc.dram_tensor("wv", [C, CPC], bf16, kind="ExternalInput").ap(),
        wp=nc.dram_tensor("wp", [CPC, C], bf16, kind="ExternalInput").ap(),
        bqk=nc.dram_tensor("bqk", [2 * CPC], f32, kind="ExternalInput").ap(),
        bv=nc.dram_tensor("bv", [CPC], f32, kind="ExternalInput").ap(),
        bp=nc.dram_tensor("bp", [C], f32, kind="ExternalInput").ap(),
        out=nc.dram_tensor("out", [T, C], bf16, kind="ExternalOutput").ap(),
    )

    with tile.TileContext(nc) as tc, contextlib.ExitStack() as ctx:
        pools = dict(
            consts=ctx.enter_context(tc.tile_pool(name="consts", bufs=1)),
            big=ctx.enter_context(tc.tile_pool(name="big", bufs=1)),
            ppool=ctx.enter_context(tc.tile_pool(name="ppool", bufs=cfg["ppool_bufs"])),
            zpool=ctx.enter_context(tc.tile_pool(name="zpool", bufs=cfg["zpool_bufs"])),
            rpool=ctx.enter_context(tc.tile_pool(name="rpool", bufs=cfg["rpool_bufs"])),
            ps_mm=ctx.enter_context(tc.tile_pool(name="ps_mm", bufs=cfg["mm_bufs"], space="PSUM")),
            ps_s=ctx.enter_context(tc.tile_pool(name="ps_s", bufs=cfg["s_bufs"], space="PSUM")),
            ps_o=ctx.enter_context(tc.tile_pool(name="ps_o", bufs=cfg["o_bufs"], space="PSUM")),
            dram=ctx.enter_context(tc.tile_pool(name="dram", bufs=2, space="DRAM")),
        )
        state = _emit_consts(nc, mybir, aps, pools)
        for _rep in range(cfg["loop"]):
            _emit_body(nc, mybir, aps, pools, state, cfg)

    nc.compile()
    return nc


def _emit_consts(nc, mybir, aps, pools):
    f32 = mybir.dt.float32
    bf16 = mybir.dt.bfloat16
    Alu = mybir.AluOpType
    consts, big = pools["consts"], pools["big"]

    # One DMA per tensor (each dma_start costs ~0.6us of sequencer time
    # plus ~1.2us fixed latency), spread across both HWDGE queues: SP gets
    # the v-path (wv + xT chunk 0), ACT gets the qk path concurrently.
    wv_sb = consts.tile([P, KC, CPC], bf16)
    nc.sync.dma_start(wv_sb, aps["wv"].rearrange("(o p) m -> p o m", p=P))
    xT_sb = big.tile([P, KC, T], bf16)
    xT_r = aps["xT"].rearrange("(o p) t -> p o t", p=P)
    nc.sync.dma_start(xT_sb[:, :, 0:QB], xT_r[:, :, 0:QB])
    wqk_sb = consts.tile([P, KC, 2 * CPC], bf16)
    nc.sync.dma_start(wqk_sb, aps["wqk"].rearrange("(o p) m -> p o m", p=P))
    bqk_sb = consts.tile([P, 2 * CPC // P], f32)
    nc.sync.dma_start(bqk_sb, aps["bqk"].rearrange("(o p) -> p o", p=P))
    wp_sb = consts.tile([P, CPC // P, C], bf16)
    nc.sync.dma_start(wp_sb, aps["wp"].rearrange("(o p) m -> p o m", p=P))
    bv_row = consts.tile([1, CPC], f32)
    nc.sync.dma_start(bv_row, aps["bv"][None, :])
    bv_bc = consts.tile([P, CPC], f32)
    nc.gpsimd.partition_broadcast(bv_bc, bv_row)
    bp_row = consts.tile([1, C], f32)
    nc.sync.dma_start(bp_row, aps["bp"][None, :])
    bp_bc = consts.tile([P, C], f32)
    nc.gpsimd.partition_broadcast(bp_bc, bp_row)

    # multiplicative causal masks for the diagonal-block offsets:
    # masks[r, p, c] = 1.0 if c >= 128*p + r else 0.0   (c within the qblock)
    masks = consts.tile([P, 4, QB], bf16)
    nc.vector.memset(masks, 1.0)
    for pos in range(4):
        nc.gpsimd.affine_select(
            out=masks[:, pos, :],
            in_=masks[:, pos, :],
            pattern=[[1, QB]],
            compare_op=Alu.is_ge,
            fill=0.0,
            base=-P * pos,
            channel_multiplier=-1,
        )

    ones64 = consts.tile([1, 64], f32)
    nc.vector.memset(ones64, 1.0)

    # warm the exp table set (~2.7us load) while DMAs stream in
    warm = consts.tile([1, 1], f32)
    nc.vector.memset(warm, 0.0)
    warm2 = consts.tile([1, 1], f32)
    nc.scalar.activation(warm2, warm, mybir.ActivationFunctionType.Exp)

    qk_sb = big.tile([P, 4, T], bf16)   # mi 0-1: q heads, 2-3: k heads
    y_sb = big.tile([P, CPC // P, T], bf16)
    v_sb = big.tile([P, NTT, HPC, 66], bf16)  # [.., 0:64]=v, [.., 64]=1.0
    nc.vector.memset(v_sb[:, :, :, 64:65], 1.0)

    return dict(
        wqk_sb=wqk_sb, wv_sb=wv_sb, wp_sb=wp_sb, bqk_sb=bqk_sb,
        bv_bc=bv_bc, bp_bc=bp_bc, masks=masks, ones64=ones64,
        xT_sb=xT_sb, qk_sb=qk_sb, y_sb=y_sb, v_sb=v_sb,
    )


def _emit_body(nc, mybir, aps, pools, st, cfg):
    f32 = mybir.dt.float32
    bf16 = mybir.dt.bfloat16
    Alu = mybir.AluOpType
    Act = mybir.ActivationFunctionType
    ppool, zpool, rpool = pools["ppool"], pools["zpool"], pools["rpool"]
    ps_mm, ps_s, ps_o, dram = pools["ps_mm"], pools["ps_s"], pools["ps_o"], pools["dram"]
    wqk_sb, wv_sb, wp_sb = st["wqk_sb"], st["wv_sb"], st["wp_sb"]
    bqk_sb, bv_bc, bp_bc, masks = st["bqk_sb"], st["bv_bc"], st["bp_bc"], st["masks"]
    xT_sb, qk_sb, y_sb, v_sb = st["xT_sb"], st["qk_sb"], st["y_sb"], st["v_sb"]
    out = aps["out"]
    xT_r = aps["xT"].rearrange("(o p) t -> p o t", p=P)

    # ---------- emission units ----------
    def xdma_unit(tc_i):
        def emit():
            tsl = slice(tc_i * QB, (tc_i + 1) * QB)
            nc.sync.dma_start(xT_sb[:, :, tsl], xT_r[:, :, tsl])
        return emit

    def qk_unit(tc_i, mi):
        def emit():
            tsl = slice(tc_i * QB, (tc_i + 1) * QB)
            ps_qk = ps_mm.tile([P, QB], f32, tag="mm", name="ps_qk")
            for ci in range(KC):
                nc.tensor.matmul(
                    ps_qk,
                    wqk_sb[:, ci, mi * P : (mi + 1) * P],
                    xT_sb[:, ci, tsl],
                    start=(ci == 0),
                    stop=(ci == KC - 1),
                )
            nc.vector.tensor_scalar_add(
                qk_sb[:, mi, tsl], ps_qk, bqk_sb[:, mi : mi + 1]
            )
        return emit

    def v_unit(tt):
        def emit():
            ps_v = ps_mm.tile([P, CPC], f32, tag="mm", name="ps_v")
            for ci in range(KC):
                nc.tensor.matmul(
                    ps_v,
                    xT_sb[:, ci, tt * P : (tt + 1) * P],
                    wv_sb[:, ci, :],
                    start=(ci == 0),
                    stop=(ci == KC - 1),
                )
            for h in range(HPC):
                nc.vector.tensor_tensor(
                    v_sb[:, tt, h, 0:64],
                    ps_v[:, h * HS : (h + 1) * HS],
                    bv_bc[:, h * HS : (h + 1) * HS],
                    Alu.add,
                )
        return emit

    z_ts = {}

    def proj_unit(qi, tl, n, z_loc, split):
        def emit():
            tt = qi * 4 + tl
            ps_z = ps_mm.tile([P, QB], f32, tag="mm", name="ps_z")
            for kc2 in range(CPC // P):
                nc.tensor.matmul(
                    ps_z,
                    y_sb[:, kc2, tt * P : (tt + 1) * P],
                    wp_sb[:, kc2, n * QB : (n + 1) * QB],
                    start=(kc2 == 0),
                    stop=(kc2 == CPC // P - 1),
                )
            if n == 0:
                z_ts[tt] = zpool.tile([P, C], bf16, tag="z", name="z_t")
            z_t = z_ts[tt]
            nc.vector.tensor_tensor(
                z_t[:, n * QB : (n + 1) * QB], ps_z,
                bp_bc[:, n * QB : (n + 1) * QB], Alu.add
            )
            if n == 1:
                nc.sync.dma_start(z_loc[tl * P : (tl + 1) * P, :], z_t)
                del z_ts[tt]
                if split and tl % 2 == 1:
                    _reduce_out(nc, mybir, pools, cfg,
                                z_loc[(tl - 1) * P : (tl + 1) * P, :],
                                out[(tt - 1) * P : (tt + 1) * P, :], 2 * P)
        return emit

    def attn_groups(qi):
        """Yield closures; each handles one kblock group for one head."""
        qsl = slice(qi * QB, (qi + 1) * QB)
        kmax = 4 * qi + 4
        # groups: lists of (kb, col_offset_in_tile, width); diagonal tiles
        # are restricted to their live columns and paired to share one exp
        groups = []
        kb = 0
        while kb < kmax:
            if cfg["pair_exp"] and kb + 1 < kmax:
                w0 = QB - P * max(0, kb - 4 * qi)
                w1 = QB - P * max(0, kb + 1 - 4 * qi)
                if w0 + w1 <= 2 * QB:
                    groups.append([(kb, 0, w0), (kb + 1, w0, w1)])
                    kb += 2
                    continue
            w0 = QB - P * max(0, kb - 4 * qi)
            groups.append([(kb, 0, w0)])
            kb += 1

        o_ts = {}

        def head_group(h, subs):
            def emit():
                po = 64 * (h % 2)
                kT = qk_sb[po : po + 64, 2 + h // 2, :]
                qT = qk_sb[po : po + 64, h // 2, qsl]
                if h not in o_ts:
                    o_ts[h] = ps_o.tile([65, QB], f32, tag="o", name="o_t")
                o_t = o_ts[h]
                tot = subs[-1][1] + subs[-1][2]
                s_t = ps_s.tile([P, 2 * QB], f32, tag="s", name="s_t")
                for kb, off, w in subs:
                    nc.tensor.matmul(
                        s_t[:, off : off + w],
                        kT[:, kb * P : (kb + 1) * P],
                        qT[:, QB - w :],
                        start=True,
                        stop=True,
                    )
                p_t = ppool.tile([P, 2 * QB], bf16, tag="p2", name="p_t")
                nc.scalar.activation(
                    p_t[:, :tot], s_t[:, :tot], Act.Exp, scale=0.125
                )
                for kb, off, w in subs:
                    pos = kb - 4 * qi
                    if pos >= 0:
                        nc.vector.tensor_tensor(
                            p_t[:, off : off + w],
                            p_t[:, off : off + w],
                            masks[:, pos, P * pos :],
                            Alu.mult,
                        )
                for kb, off, w in subs:
                    nc.tensor.matmul(
                        o_t[:, QB - w :],
                        v_sb[:, kb, h, 0:65],
                        p_t[:, off : off + w],
                        start=(kb == 0),
                        stop=(kb == kmax - 1),
                    )
            return emit

        def finisher(h):
            def emit():
                po = 64 * (h % 2)
                r_t = rpool.tile([1, QB], f32, tag="r", name="r_t")
                nc.vector.reciprocal(r_t, o_ts[h][64:65, :])
                if cfg["pe_bcast"]:
                    rb_p = ps_s.tile([64, QB], f32, tag="s", name="rb_p")
                    nc.tensor.matmul(rb_p, st["ones64"], r_t, start=True, stop=True)
                    nc.vector.tensor_tensor(
                        y_sb[po : po + 64, h // 2, qsl], o_ts[h][0:64, :], rb_p,
                        Alu.mult,
                    )
                else:
                    rb_t = rpool.tile([64, QB], f32, tag="rb", name="rb_t")
                    nc.gpsimd.partition_broadcast(rb_t, r_t)
                    nc.vector.tensor_tensor(
                        y_sb[po : po + 64, h // 2, qsl], o_ts[h][0:64, :], rb_t,
                        Alu.mult,
                    )
                del o_ts[h]
            return emit

        il = cfg["interleave"]
        units = []
        for hp in range(HPC // il):
            heads = tuple(range(il * hp, il * hp + il))
            for subs in groups:
                for h in heads:
                    units.append(head_group(h, subs))
            for h in heads:
                units.append(finisher(h))
        return units

    # ---------- merged schedule ----------
    # attention(qi) groups interleaved with filler units (qkv of chunk
    # qi+1, proj of block qi-1) so the in-order PE queue never starves on
    # the S->exp->PV dependency chain.
    def qkv_units(tc_i):
        u = []
        if tc_i > 0:
            u.append(xdma_unit(tc_i))
        vu = [v_unit(tc_i * 4 + tl) for tl in range(4)]
        qu = [qk_unit(tc_i, mi) for mi in range(4)]
        # chunk 0: v first -- wv + xT chunk 0 are the first DMAs to land
        return u + (vu + qu if tc_i == 0 else qu + vu)

    z_locs = {}

    def proj_units(qi, split):
        z_locs[qi] = dram.tile([QB, C], bf16, tag="zloc", name="z_loc")
        u = []
        for tl in range(4):
            for n in range(2):
                u.append(proj_unit(qi, tl, n, z_locs[qi], split))
        return u

    def finish_block(qi, split):
        if not split:
            def emit():
                _reduce_out(nc, mybir, pools, cfg, z_locs[qi],
                            out[qi * QB : (qi + 1) * QB, :], QB)
            return [emit]
        return []

    if not cfg["merged"]:
        for tc_i in range(NQ):
            for u in qkv_units(tc_i):
                u()
        for qi_idx, qi in enumerate(
            [(cfg["qi_first"] + i) % NQ for i in range(NQ)]
        ):
            split = cfg["tail_split"] and qi_idx == NQ - 1
            for u in attn_groups(qi):
                u()
            for u in proj_units(qi, split) + finish_block(qi, split):
                u()
        return

    # merged: xT DMAs up front, then qi rounds with fillers woven in
    for u in qkv_units(0):
        u()
    for qi in range(NQ):
        split = cfg["tail_split"] and qi == NQ - 1
        att = attn_groups(qi)
        fillers = []
        if qi + 1 < NQ:
            fillers += qkv_units(qi + 1)
        if qi > 0:
            fillers += proj_units(qi - 1, False) + finish_block(qi - 1, False)
        # weave fillers evenly among attention groups
        n_att, n_fill = len(att), len(fillers)
        fi = 0
        bias = cfg["weave_bias"]
        for gi, u in enumerate(att):
            u()
            want = int((((gi + 1) / n_att) ** bias) * n_fill)
            while fi < want:
                fillers[fi]()
                fi += 1
        while fi < n_fill:
            fillers[fi]()
            fi += 1
    for u in proj_units(NQ - 1, split) + finish_block(NQ - 1, split):
        u()


def _reduce_out(nc, mybir, pools, cfg, z_loc_ap, out_ap, rows):
    f32 = mybir.dt.float32
    Alu = mybir.AluOpType
    bf16 = mybir.dt.bfloat16
    if cfg["with_cc"]:
        z_red = pools["dram"].tile([rows, C], bf16, tag=f"zred{rows}")
        nc.gpsimd.collective_compute(
            "AllReduce",
            Alu.add,
            replica_groups=GROUPS,
            ins=[z_loc_ap.opt()],
            outs=[z_red.opt()],
        )
        nc.sync.dma_start(out_ap, z_red)
    else:
        nc.sync.dma_start(out_ap, z_loc_ap)


def get_nc(**overrides):
    cfg = dict(DEFAULT_CFG)
    cfg.update(overrides)
    key = tuple(sorted(cfg.items()))
    if key not in _CACHE:
        _CACHE[key] = _build_nc(cfg)
    return _CACHE[key]


def make_in_maps(x, w_attn, b_attn, w_proj, b_proj):
    x = np.asarray(x, dtype=np.float32)
    w_attn = np.asarray(w_attn, dtype=np.float32)
    b_attn = np.asarray(b_attn, dtype=np.float32)
    w_proj = np.asarray(w_proj, dtype=np.float32)
    b_proj = np.asarray(b_proj, dtype=np.float32)
    bf = ml_dtypes.bfloat16

    in_maps = []
    for core in range(NCORES):
        b, g = core // 4, core % 4
        hsl = slice(g * CPC, (g + 1) * CPC)
        wq = w_attn[:, 0:C][:, hsl]
        wk = w_attn[:, C : 2 * C][:, hsl]
        wv_ = w_attn[:, 2 * C : 3 * C][:, hsl]
        in_maps.append(
            {
                "xT": np.ascontiguousarray(x[b].T).astype(bf),
                "wqk": np.ascontiguousarray(np.concatenate([wq, wk], axis=1)).astype(bf),
                "wv": np.ascontiguousarray(wv_).astype(bf),
                "wp": np.ascontiguousarray(w_proj[hsl, :]).astype(bf),
                "bqk": np.concatenate(
                    [b_attn[0:C][hsl], b_attn[C : 2 * C][hsl]]
                ).astype(np.float32),
                "bv": np.ascontiguousarray(b_attn[2 * C : 3 * C][hsl]).astype(np.float32),
                # every core in a reduce group adds its bp share pre-AllReduce
                "bp": (b_proj / 4.0).astype(np.float32),
            }
        )
    return in_maps


def kernel(x, w_attn, b_attn, w_proj, b_proj):
    from concourse.bass_utils import run_bass_kernel_spmd

    nc = get_nc()
    in_maps = make_in_maps(x, w_attn, b_attn, w_proj, b_proj)
    res = run_bass_kernel_spmd(nc, in_maps, core_ids=list(range(NCORES))).results
    out = np.empty((B, T, C), np.float32)
    out[0] = res[0]["out"].astype(np.float32)
    out[1] = res[4]["out"].astype(np.float32)
    return out

